# revision 24
# baseline (speedup 1.0000x reference)
"""PointNet MLP (3 x conv1x1+BN+ReLU, final valid-mask) on 8 TRN2 cores.

Sharding: compacted-column parallel. The valid mask keeps ~70% of the
4096*128 = 524288 point-neighbor columns; masked columns are exactly 0 in
the reference output. Host gathers the valid columns, splits them evenly
across 8 cores, device computes only those, host scatters into zeros.

Numerics: plain fp16 matmuls with f32 PSUM accumulation, fp16 output
upcast on host (harness gate is rel_err < 2e-2; this lands ~1e-3).

Device: software-pipelined (modulo) schedule, slots of 2048 data cols
(blocks A|B of M=1024 packed on 128 partitions for L1/L2):
  stage0 (slot t):   mm1(t) K=6 -> ps12 ; hi1(t) = ACT Relu+b1 -> fp16
  stage1 (slot t+1): mm2(t) K=128 -> ps12 (same buf) ; hi2(t) = DVE
  stage2 (slot t+3): mm3 4x K=64 quarters -> ps3 [128,2048] ;
                     drains split ACT [0:XA] / DVE [XA:2048] -> fp16 ob
                     -> one 512KB DMA
PSUM: ps12 pool bufs=2 (4 banks) + ps3 bufs=1 (4 banks) = 8 banks.
XA=1228 balances ACT (0.81 ns/col) vs DVE (0.98 ns/col) drain loads;
per-slot engine busy ~2.37us each, DVE/ACT-bound -> ~58-63us predicted.
"""

import numpy as np

try:
    import concourse.bass as bass
except ImportError:
    import sys

    sys.path.insert(0, "/opt/trn_rl_repo")
    import concourse.bass as bass

import concourse.bacc as bacc

import concourse.mybir as mybir
from concourse import tile
from concourse.bass_utils import run_bass_kernel_spmd

F32 = mybir.dt.float32
F16 = mybir.dt.float16

N_CORES = 8
NPOINT, KNN = 4096, 128
NCOLS = NPOINT * KNN
M = 1024
XA = 1228
S1, S2 = 1, 3
EPS = 1e-5

_NC_CACHE = {}


def _build_nc(iters):
    nc = bacc.Bacc("TRN2", target_bir_lowering=False)
    xp_d = nc.declare_dram_parameter("xp", [6, iters * M], F16, isOutput=False)
    w1_d = nc.declare_dram_parameter("lhsT1", [6, 64], F16, isOutput=False)
    w2_d = nc.declare_dram_parameter("lhsT2", [128, 128], F16, isOutput=False)
    w3_d = nc.declare_dram_parameter("lhsT3", [128, 128], F16, isOutput=False)
    bias_d = nc.declare_dram_parameter("biases", [128, 3], F32, isOutput=False)
    out_d = nc.declare_dram_parameter("out", [128, iters * 2 * M], F16, isOutput=True)

    add = mybir.AluOpType.add
    vmax = mybir.AluOpType.max
    relu_fn = mybir.ActivationFunctionType.Relu
    H = M // 2

    with tile.TileContext(nc) as tc:
        with (
            tc.tile_pool(name="const", bufs=1) as cpool,
            tc.tile_pool(name="xpool", bufs=1) as xpool,
            tc.tile_pool(name="h1pool", bufs=3) as h1pool,
            tc.tile_pool(name="h2pool", bufs=4) as h2pool,
            tc.tile_pool(name="opool", bufs=4) as opool,
            tc.tile_pool(name="ps12", bufs=2, space="PSUM") as ps12pool,
            tc.tile_pool(name="ps3", bufs=1, space="PSUM") as ps3pool,
        ):
            # block-A weights/x on row strips 0-1 (partitions 0..), block-B
            # on strips 2-3 (partitions 32+/64+) so the per-block matmuls
            # occupy disjoint PE row groups and co-execute.
            w1_sb = cpool.tile([35, 64], F16, tag="w1")
            w2_sb = cpool.tile([128, 128], F16, tag="w2")
            w3_sb = cpool.tile([128, 128], F16, tag="w3")
            bias_sb = cpool.tile([128, 3], F32, tag="bias")
            x_sb = xpool.tile([35, iters * M], F16, tag="x")
            # small weight DMAs first (they'd otherwise queue behind the
            # big x transfers on shared rings), then x in chunks so early
            # slots start early
            nc.sync.dma_start(w1_sb[0:3, :], w1_d[0:3, :])
            nc.sync.dma_start(w1_sb[32:35, :], w1_d[3:6, :])
            nc.sync.dma_start(w2_sb[:, :], w2_d[:, :])
            nc.sync.dma_start(w3_sb[:, :], w3_d[:, :])
            nc.sync.dma_start(bias_sb[:, :], bias_d[:, :])
            NX = iters * M
            lo = 0
            for hi in sorted({min(c, NX) for c in (4 * M, 10 * M, NX)}):
                if hi > lo:
                    nc.sync.dma_start(x_sb[0:3, lo:hi], xp_d[0:3, lo:hi])
                    nc.sync.dma_start(x_sb[32:35, lo:hi], xp_d[3:6, lo:hi])
                lo = hi
            b1_ap = bias_sb[:, 0:1]
            b2_ap = bias_sb[:, 1:2]
            b3_ap = bias_sb[:, 2:3]

            # HAM warm-up: ~7us of dense back-to-back small matmuls flip
            # the PE clock gate to 2.4 GHz before the pipeline starts
            # (promotion needs near-continuous PE duty; the steady-state
            # burst pattern alone leaves it at 1.2 GHz, and a multi-us
            # tensor stall right after warm-up demotes it again — so the
            # first slots' L1 work is interleaved into the dummy stream).
            # Fed from a memset tile so it does not wait on any DMA.
            # Scratch = the ps3 storage, free until stage2 of slot 0.
            wsrc = cpool.tile([128, 256], F16, tag="wsrc")
            nc.vector.memset(wsrc[:, :], 0.0)
            scratch = ps3pool.tile([128, M], F32, tag="ps3b", name="warm")

            def dummies(n):
                for i in range(n):
                    r = (i % 8) * 128
                    nc.tensor.matmul(scratch[:, r : r + 128],
                                     wsrc[:, 0:128], wsrc[:, 0:128])

            dummies(40)

            ps12 = {}
            hi1 = {}
            hi2 = {}

            for t in range(iters + S2):
                # ---- stage2: L3 matmuls + drains + DMA for slot k ----
                k = t - S2
                if 0 <= k < iters:
                    h2 = hi2.pop(k)
                    # two decoupled psum tiles -> two parallel ps3-reuse
                    # chains (mm3x(k) only waits its own drain of k-1)
                    ps3b = ps3pool.tile([128, M], F32, tag="ps3b", name="ps3b")
                    ps3a = ps3pool.tile([128, M], F32, tag="ps3a", name="ps3a")
                    # heartbeat: keeps PE duty high enough to hold the
                    # 2.4 GHz clock gate; overwritten by the real matmul
                    nc.tensor.matmul(ps3b[:, 0:256], wsrc[:, 0:128],
                                     wsrc[:, 0:256])
                    nc.tensor.matmul(ps3b[:, 0:H],
                                     w3_sb[64:128, :], h2[64:128, 0:H])
                    nc.tensor.matmul(ps3b[:, H:M],
                                     w3_sb[64:128, :], h2[64:128, H:M])
                    nc.tensor.matmul(ps3a[:, 0:H], w3_sb[0:64, :], h2[0:64, 0:H])
                    nc.tensor.matmul(ps3a[:, H:M], w3_sb[0:64, :], h2[0:64, H:M])
                    ob = opool.tile([128, 2 * M], F16, tag="ob", name="ob")
                    nc.vector.tensor_scalar(
                        ob[:, M : 2 * M], ps3b[:, :], b3_ap, 0.0, add, vmax,
                    )
                    nc.scalar.activation(ob[:, 0:M], ps3a[:, :],
                                         relu_fn, bias=b3_ap)
                    nc.sync.dma_start(out_d[:, 2 * M * k : 2 * M * (k + 1)],
                                      ob[:, :])

                # ---- stage0: L1 matmul + ACT drain for slot t ----
                if t < iters:
                    c0 = t * M
                    p = ps12pool.tile([128, M], F32, tag="ps12", name="ps12")
                    ps12[t] = p
                    if t >= 2:
                        nc.tensor.matmul(p[:, 0:256], wsrc[:, 0:128],
                                         wsrc[:, 0:256])
                    for h in range(2):
                        s = slice(h * H, (h + 1) * H)
                        cs = slice(c0 + h * H, c0 + (h + 1) * H)
                        nc.tensor.matmul(p[0:64, s], w1_sb[0:3, :],
                                         x_sb[0:3, cs])
                        nc.tensor.matmul(p[64:128, s], w1_sb[32:35, :],
                                         x_sb[32:35, cs])
                    h1 = h1pool.tile([128, M], F16, tag="hi1", name="hi1")
                    hi1[t] = h1
                    nc.scalar.activation(h1[:, :], p[:, :], relu_fn, bias=b1_ap)
                    if t < 2:
                        # keep the PE clock-gate promotion going while the
                        # pipeline fills (no multi-us tensor idle allowed)
                        dummies(12)

                # ---- stage1: L2 matmul + DVE drain for slot t-1 ----
                k = t - S1
                if 0 <= k < iters:
                    p = ps12.pop(k)
                    h1 = hi1.pop(k)
                    for h in range(2):
                        s = slice(h * H, (h + 1) * H)
                        nc.tensor.matmul(p[0:64, s], w2_sb[0:64, 0:64],
                                         h1[0:64, s])
                        nc.tensor.matmul(p[64:128, s], w2_sb[64:128, 64:128],
                                         h1[64:128, s])
                    h2 = h2pool.tile([128, M], F16, tag="hi2", name="hi2")
                    hi2[k] = h2
                    nc.vector.tensor_scalar(h2[:, :], p[:, :],
                                            b2_ap, 0.0, add, vmax)

    nc.compile()
    return nc


def _get_nc(iters):
    if iters not in _NC_CACHE:
        _NC_CACHE[iters] = _build_nc(iters)
    return _NC_CACHE[iters]


def _fold_bn(W, b, gamma, beta, mean, var):
    inv = gamma.astype(np.float64) / np.sqrt(var.astype(np.float64) + EPS)
    Wp = (W.astype(np.float64) * inv[:, None]).astype(np.float32)
    bp = ((b.astype(np.float64) - mean.astype(np.float64)) * inv
          + beta.astype(np.float64)).astype(np.float32)
    return Wp, bp


def _prepare(inputs):
    gp = np.asarray(inputs["grouped_pc"], dtype=np.float32)
    valid = np.asarray(inputs["valid"], dtype=np.float32)

    Wp1, bp1 = _fold_bn(*(np.asarray(inputs[k], dtype=np.float32)
                          for k in ("W1", "b1", "gamma1", "beta1", "mean1", "var1")))
    Wp2, bp2 = _fold_bn(*(np.asarray(inputs[k], dtype=np.float32)
                          for k in ("W2", "b2", "gamma2", "beta2", "mean2", "var2")))
    Wp3, bp3 = _fold_bn(*(np.asarray(inputs[k], dtype=np.float32)
                          for k in ("W3", "b3", "gamma3", "beta3", "mean3", "var3")))

    lhsT1 = np.zeros((6, 64), np.float16)
    lhsT1[0:3, :] = Wp1.T.astype(np.float16)
    lhsT1[3:6, :] = Wp1.T.astype(np.float16)

    lhsT2 = np.zeros((128, 128), np.float16)
    lhsT2[0:64, 0:64] = Wp2.T.astype(np.float16)
    lhsT2[64:128, 64:128] = Wp2.T.astype(np.float16)

    lhsT3 = np.zeros((128, 128), np.float16)
    lhsT3[0:64, :] = Wp3.T.astype(np.float16)
    lhsT3[64:128, :] = Wp3.T.astype(np.float16)

    biases = np.zeros((128, 3), np.float32)
    biases[:, 0] = np.concatenate([bp1, bp1])
    biases[:, 1] = np.concatenate([bp2, bp2])
    biases[:, 2] = bp3

    x = gp[0].reshape(3, NCOLS)
    vidx = np.flatnonzero(valid.reshape(NCOLS) > 0.5)
    V = len(vidx)
    Vc = -(-V // N_CORES)
    iters = max(1, -(-Vc // (2 * M)))
    cap = iters * 2 * M

    xv = x[:, vidx].astype(np.float16)

    in_maps = []
    for c in range(N_CORES):
        lo_i = c * Vc
        hi_i = min((c + 1) * Vc, V)
        n = max(0, hi_i - lo_i)
        a = np.zeros((3, cap), np.float16)
        if n:
            a[:, :n] = xv[:, lo_i:hi_i]
        ar = a.reshape(3, iters, 2, M)
        xp = np.empty((6, iters * M), np.float16)
        xp[0:3] = ar[:, :, 0, :].reshape(3, -1)
        xp[3:6] = ar[:, :, 1, :].reshape(3, -1)
        in_maps.append(
            {
                "xp": np.ascontiguousarray(xp),
                "lhsT1": lhsT1,
                "lhsT2": lhsT2,
                "lhsT3": lhsT3,
                "biases": biases,
            }
        )
    return in_maps, vidx, V, Vc, iters


def _gather(results, vidx, V, Vc):
    stream = np.empty((128, V), np.float32)
    for c in range(N_CORES):
        lo_i = c * Vc
        hi_i = min((c + 1) * Vc, V)
        if hi_i <= lo_i:
            break
        stream[:, lo_i:hi_i] = results[c]["out"][:, : hi_i - lo_i].astype(np.float32)
    full = np.zeros((128, NCOLS), np.float32)
    full[:, vidx] = stream
    return full.reshape(128, NPOINT, KNN)[None]


def run_traced(trace=False, **inputs):
    in_maps, vidx, V, Vc, iters = _prepare(inputs)
    nc = _get_nc(iters)
    res = run_bass_kernel_spmd(nc, in_maps, list(range(N_CORES)), trace=trace)
    return _gather(res.results, vidx, V, Vc), res.exec_time_ns


def kernel(**inputs):
    out, _ = run_traced(trace=False, **inputs)
    return out


# revision 28
# speedup vs baseline: 1.0990x; 1.0990x over previous
"""PointNet MLP (3 x conv1x1+BN+ReLU, final valid-mask) on 8 TRN2 cores.

Sharding: compacted-column parallel. The valid mask keeps ~70% of the
4096*128 = 524288 point-neighbor columns; masked columns are exactly 0 in
the reference output. Host gathers the valid columns, splits them evenly
across 8 cores, device computes only those, host scatters into zeros.

Numerics: plain fp16 matmuls with f32 PSUM accumulation, fp16 output
upcast on host (harness gate is rel_err < 2e-2; this lands ~1e-3).

Device: software-pipelined (modulo) schedule, slots of 2048 data cols
(blocks A|B of M=1024 packed on 128 partitions for L1/L2):
  stage0 (slot t):   mm1(t) K=6 -> ps12 ; hi1(t) = ACT Relu+b1 -> fp16
  stage1 (slot t+1): mm2(t) K=128 -> ps12 (same buf) ; hi2(t) = DVE
  stage2 (slot t+3): mm3 4x K=64 quarters -> ps3 [128,2048] ;
                     drains split ACT [0:XA] / DVE [XA:2048] -> fp16 ob
                     -> one 512KB DMA
PSUM: ps12 pool bufs=2 (4 banks) + ps3 bufs=1 (4 banks) = 8 banks.
XA=1228 balances ACT (0.81 ns/col) vs DVE (0.98 ns/col) drain loads;
per-slot engine busy ~2.37us each, DVE/ACT-bound -> ~58-63us predicted.
"""

import numpy as np

try:
    import concourse.bass as bass
except ImportError:
    import sys

    sys.path.insert(0, "/opt/trn_rl_repo")
    import concourse.bass as bass

import concourse.bacc as bacc

import concourse.mybir as mybir
from concourse import tile
from concourse.bass_utils import run_bass_kernel_spmd

F32 = mybir.dt.float32
F16 = mybir.dt.float16

N_CORES = 8
NPOINT, KNN = 4096, 128
NCOLS = NPOINT * KNN
M = 1024
XA = 1228
S1, S2 = 1, 3
EPS = 1e-5

_NC_CACHE = {}


def _build_nc(iters):
    nc = bacc.Bacc("TRN2", target_bir_lowering=False)
    xp_d = nc.declare_dram_parameter("xp", [6, iters * M], F16, isOutput=False)
    w1_d = nc.declare_dram_parameter("lhsT1", [6, 64], F16, isOutput=False)
    w2_d = nc.declare_dram_parameter("lhsT2", [128, 128], F16, isOutput=False)
    w3_d = nc.declare_dram_parameter("lhsT3", [128, 128], F16, isOutput=False)
    bias_d = nc.declare_dram_parameter("biases", [128, 3], F32, isOutput=False)
    out_d = nc.declare_dram_parameter("out", [128, iters * 2 * M], F16, isOutput=True)

    add = mybir.AluOpType.add
    vmax = mybir.AluOpType.max
    relu_fn = mybir.ActivationFunctionType.Relu
    H = M // 2

    with tile.TileContext(nc) as tc:
        with (
            tc.tile_pool(name="const", bufs=1) as cpool,
            tc.tile_pool(name="xpool", bufs=1) as xpool,
            tc.tile_pool(name="h1pool", bufs=3) as h1pool,
            tc.tile_pool(name="h2pool", bufs=4) as h2pool,
            tc.tile_pool(name="opool", bufs=4) as opool,
            tc.tile_pool(name="ps12", bufs=2, space="PSUM") as ps12pool,
            tc.tile_pool(name="ps3", bufs=1, space="PSUM") as ps3pool,
        ):
            # block-A weights/x on row strips 0-1 (partitions 0..), block-B
            # on strips 2-3 (partitions 32+/64+) so the per-block matmuls
            # occupy disjoint PE row groups and co-execute.
            w1_sb = cpool.tile([35, 64], F16, tag="w1")
            w2_sb = cpool.tile([128, 128], F16, tag="w2")
            w3_sb = cpool.tile([128, 128], F16, tag="w3")
            bias_sb = cpool.tile([128, 3], F32, tag="bias")
            x_sb = xpool.tile([35, iters * M], F16, tag="x")
            # small weight DMAs first (they'd otherwise queue behind the
            # big x transfers on shared rings), then x in chunks so early
            # slots start early
            nc.sync.dma_start(w1_sb[0:3, :], w1_d[0:3, :])
            nc.sync.dma_start(w1_sb[32:35, :], w1_d[3:6, :])
            nc.sync.dma_start(w2_sb[:, :], w2_d[:, :])
            nc.sync.dma_start(w3_sb[:, :], w3_d[:, :])
            nc.sync.dma_start(bias_sb[:, :], bias_d[:, :])
            NX = iters * M
            lo = 0
            for hi in sorted({min(c, NX) for c in (4 * M, 10 * M, NX)}):
                if hi > lo:
                    nc.sync.dma_start(x_sb[0:3, lo:hi], xp_d[0:3, lo:hi])
                    nc.sync.dma_start(x_sb[32:35, lo:hi], xp_d[3:6, lo:hi])
                lo = hi
            b1_ap = bias_sb[:, 0:1]
            b2_ap = bias_sb[:, 1:2]
            b3_ap = bias_sb[:, 2:3]

            ps12 = {}
            hi1 = {}
            hi2 = {}

            for t in range(iters + S2):
                # ---- stage2: L3 matmuls + drains + DMA for slot k ----
                k = t - S2
                if 0 <= k < iters:
                    h2 = hi2.pop(k)
                    # two decoupled psum tiles -> two parallel ps3-reuse
                    # chains (mm3x(k) only waits its own drain of k-1)
                    ps3b = ps3pool.tile([128, M], F32, tag="ps3b", name="ps3b")
                    ps3a = ps3pool.tile([128, M], F32, tag="ps3a", name="ps3a")
                    nc.tensor.matmul(ps3b[:, 0:H],
                                     w3_sb[64:128, :], h2[64:128, 0:H])
                    nc.tensor.matmul(ps3b[:, H:M],
                                     w3_sb[64:128, :], h2[64:128, H:M])
                    nc.tensor.matmul(ps3a[:, 0:H], w3_sb[0:64, :], h2[0:64, 0:H])
                    nc.tensor.matmul(ps3a[:, H:M], w3_sb[0:64, :], h2[0:64, H:M])
                    ob = opool.tile([128, 2 * M], F16, tag="ob", name="ob")
                    nc.vector.tensor_scalar(
                        ob[:, M : 2 * M], ps3b[:, :], b3_ap, 0.0, add, vmax,
                    )
                    nc.scalar.activation(ob[:, 0:M], ps3a[:, :],
                                         relu_fn, bias=b3_ap)
                    nc.sync.dma_start(out_d[:, 2 * M * k : 2 * M * (k + 1)],
                                      ob[:, :])

                # ---- stage0: L1 matmul + ACT drain for slot t ----
                if t < iters:
                    c0 = t * M
                    p = ps12pool.tile([128, M], F32, tag="ps12", name="ps12")
                    ps12[t] = p
                    for h in range(2):
                        s = slice(h * H, (h + 1) * H)
                        cs = slice(c0 + h * H, c0 + (h + 1) * H)
                        nc.tensor.matmul(p[0:64, s], w1_sb[0:3, :],
                                         x_sb[0:3, cs])
                        nc.tensor.matmul(p[64:128, s], w1_sb[32:35, :],
                                         x_sb[32:35, cs])
                    h1 = h1pool.tile([128, M], F16, tag="hi1", name="hi1")
                    hi1[t] = h1
                    nc.scalar.activation(h1[:, :], p[:, :], relu_fn, bias=b1_ap)

                # ---- stage1: L2 matmul + DVE drain for slot t-1 ----
                k = t - S1
                if 0 <= k < iters:
                    p = ps12.pop(k)
                    h1 = hi1.pop(k)
                    for h in range(2):
                        s = slice(h * H, (h + 1) * H)
                        nc.tensor.matmul(p[0:64, s], w2_sb[0:64, 0:64],
                                         h1[0:64, s])
                        nc.tensor.matmul(p[64:128, s], w2_sb[64:128, 64:128],
                                         h1[64:128, s])
                    h2 = h2pool.tile([128, M], F16, tag="hi2", name="hi2")
                    hi2[k] = h2
                    nc.vector.tensor_scalar(h2[:, :], p[:, :],
                                            b2_ap, 0.0, add, vmax)

    nc.compile()
    return nc


def _get_nc(iters):
    if iters not in _NC_CACHE:
        _NC_CACHE[iters] = _build_nc(iters)
    return _NC_CACHE[iters]


def _fold_bn(W, b, gamma, beta, mean, var):
    inv = gamma.astype(np.float64) / np.sqrt(var.astype(np.float64) + EPS)
    Wp = (W.astype(np.float64) * inv[:, None]).astype(np.float32)
    bp = ((b.astype(np.float64) - mean.astype(np.float64)) * inv
          + beta.astype(np.float64)).astype(np.float32)
    return Wp, bp


def _prepare(inputs):
    gp = np.asarray(inputs["grouped_pc"], dtype=np.float32)
    valid = np.asarray(inputs["valid"], dtype=np.float32)

    Wp1, bp1 = _fold_bn(*(np.asarray(inputs[k], dtype=np.float32)
                          for k in ("W1", "b1", "gamma1", "beta1", "mean1", "var1")))
    Wp2, bp2 = _fold_bn(*(np.asarray(inputs[k], dtype=np.float32)
                          for k in ("W2", "b2", "gamma2", "beta2", "mean2", "var2")))
    Wp3, bp3 = _fold_bn(*(np.asarray(inputs[k], dtype=np.float32)
                          for k in ("W3", "b3", "gamma3", "beta3", "mean3", "var3")))

    lhsT1 = np.zeros((6, 64), np.float16)
    lhsT1[0:3, :] = Wp1.T.astype(np.float16)
    lhsT1[3:6, :] = Wp1.T.astype(np.float16)

    lhsT2 = np.zeros((128, 128), np.float16)
    lhsT2[0:64, 0:64] = Wp2.T.astype(np.float16)
    lhsT2[64:128, 64:128] = Wp2.T.astype(np.float16)

    lhsT3 = np.zeros((128, 128), np.float16)
    lhsT3[0:64, :] = Wp3.T.astype(np.float16)
    lhsT3[64:128, :] = Wp3.T.astype(np.float16)

    biases = np.zeros((128, 3), np.float32)
    biases[:, 0] = np.concatenate([bp1, bp1])
    biases[:, 1] = np.concatenate([bp2, bp2])
    biases[:, 2] = bp3

    x = gp[0].reshape(3, NCOLS)
    vidx = np.flatnonzero(valid.reshape(NCOLS) > 0.5)
    V = len(vidx)
    Vc = -(-V // N_CORES)
    iters = max(1, -(-Vc // (2 * M)))
    cap = iters * 2 * M

    xv = x[:, vidx].astype(np.float16)

    in_maps = []
    for c in range(N_CORES):
        lo_i = c * Vc
        hi_i = min((c + 1) * Vc, V)
        n = max(0, hi_i - lo_i)
        a = np.zeros((3, cap), np.float16)
        if n:
            a[:, :n] = xv[:, lo_i:hi_i]
        ar = a.reshape(3, iters, 2, M)
        xp = np.empty((6, iters * M), np.float16)
        xp[0:3] = ar[:, :, 0, :].reshape(3, -1)
        xp[3:6] = ar[:, :, 1, :].reshape(3, -1)
        in_maps.append(
            {
                "xp": np.ascontiguousarray(xp),
                "lhsT1": lhsT1,
                "lhsT2": lhsT2,
                "lhsT3": lhsT3,
                "biases": biases,
            }
        )
    return in_maps, vidx, V, Vc, iters


def _gather(results, vidx, V, Vc):
    stream = np.empty((128, V), np.float32)
    for c in range(N_CORES):
        lo_i = c * Vc
        hi_i = min((c + 1) * Vc, V)
        if hi_i <= lo_i:
            break
        stream[:, lo_i:hi_i] = results[c]["out"][:, : hi_i - lo_i].astype(np.float32)
    full = np.zeros((128, NCOLS), np.float32)
    full[:, vidx] = stream
    return full.reshape(128, NPOINT, KNN)[None]


def run_traced(trace=False, **inputs):
    in_maps, vidx, V, Vc, iters = _prepare(inputs)
    nc = _get_nc(iters)
    res = run_bass_kernel_spmd(nc, in_maps, list(range(N_CORES)), trace=trace)
    return _gather(res.results, vidx, V, Vc), res.exec_time_ns


def kernel(**inputs):
    out, _ = run_traced(trace=False, **inputs)
    return out
